# revision 30
# baseline (speedup 1.0000x reference)
"""Causal attention kernel for Trainium2 (Bass/Tile), 8-core SPMD.

Problem: B=2, H=16, S=2048, D=64 fp32 attention with a causal mask.
Sharding: batch*heads = 32 slices -> 4 heads per core across 8 cores.

Per-core algorithm (heads processed in pairs stacked on SBUF partitions):
  S^T = K @ Q^T blockwise in bf16: [kblock=128, qtile=512] tiles, k on PSUM
  partitions, q on the free dim. The two heads of a pair use PE row tiles
  (0,0)/(64,0) (contraction D=64) so their QK^T matmuls run CONCURRENTLY
  in the two halves of the systolic array.
  exp: split across ScalarE (exact ACT exp -> bf16) and VectorE
  (Schraudolph bit-trick: bf16bits = uint16(s*128/(8 ln2) + B); the causal
  mask is FUSED as an additive per-element bias tile (masked -> negative ->
  uint16 saturates to 0 -> +0.0 bf16), one scalar_tensor_tensor op total.
  P^T tiles (bf16) persist in SBUF for a whole qtile; the AV pass
  out^T = V_aug^T @ P^T (V_aug has a ones column -> row 64 = softmax
  denominator) runs as 4 sequenced passes (headA-half0, headB-half0,
  headA-half1, headB-half1) over 64-row PE tiles, each half accumulating
  into the SAME single PSUM bank per head (pass separation prevents
  row-tile bank races). AV matmuls are emitted as FILLER between the NEXT
  qtile's QK^T groups, so PSUM st tiles live single-buffered (4 banks) and
  accs double-buffered (4 banks) = exactly 8 banks.
  Host divides by the denominator and transposes back.

All matmuls bf16 (1 cycle/row, FWL weight loads); DMA is halved vs fp32.
"""

import sys
from collections import deque

import numpy as np

for _p in ('/opt/trn_rl_repo', '/root/.axon_site/_ro/trn_rl_repo'):
    if _p not in sys.path:
        sys.path.append(_p)

B, H, S, D = 2, 16, 2048, 64
NCORES = 8
HPC = (B * H) // NCORES  # heads per core = 4
QT = 512                 # q tile (PSUM bank free dim)
KB = 128                 # k block (partition dim)
GSIZE = 2                # kblocks per exp group (ACT overhead amortization)
NQT = S // QT            # 4
NKB = S // KB            # 16
YGRAN = 128              # q-trim granularity for matmuls

# Schraudolph exp -> bf16 bits: bits = uint16(s * A_S + B_S), computed on
# raw (unscaled) scores so A_S folds the 1/sqrt(D) softmax scale.
A_S = 128.0 / (float(np.log(2.0)) * 8.0)
C_S = 4.75               # sawtooth centering (covers rint-vs-floor convert)
B_S = 127.0 * 128.0 - C_S
# bias value for masked elements: bits land in [2900, 5100] for any |s|<=47,
# i.e. a ~1e-30 bf16 (effectively 0) without relying on convert saturation
MASKED_B = 4000.0

_CACHE = {}


def _plan_from_mask(mask):
    """Classify each (qtile, kblock-group) region of the mask.

    plan[j]: tuple of groups (i0, w, kind, pat, y0g, y0k):
      kblocks [i0, i0+w), kind in {'full','mixed'}, pat indexes the deduped
      bias patterns, y0g trims the exp AP (256-granular), y0k[gp] trims the
      QK^T / AV matmuls per kblock (YGRAN-granular).
    pats[p]: ('causal', rel, w) generated on-chip, or ('data', arr) DMA'd.
    """
    plan = []
    pats = []
    pat_idx = {}
    for j in range(NQT):
        row = []
        mq = mask[j * QT:(j + 1) * QT]  # [QT, S] (q rows, k cols)
        for i0 in range(0, NKB, GSIZE):
            w = min(GSIZE, NKB - i0)
            blk = [mq[:, i * KB:(i + 1) * KB].any() for i in range(i0, i0 + w)]
            if not any(blk):
                continue
            lead = blk.index(True)
            i0 += lead
            w = len(blk) - lead - blk[::-1].index(True)
            R = mq[:, i0 * KB:(i0 + w) * KB]  # [QT, w*KB]
            y0k = []
            for gp in range(w):
                Rb = R[:, gp * KB:(gp + 1) * KB]
                live = np.nonzero(Rb.any(axis=1))[0]
                y0 = (int(live[0]) // YGRAN) * YGRAN if len(live) else 0
                y0k.append(min(y0, QT - YGRAN))
            if R.all():
                row.append((i0, w, 'full', -1, 0, tuple([0] * w)))
                continue
            y0g = min((y0k[0] // 256) * 256, QT - 256)
            qq = np.arange(j * QT, (j + 1) * QT)[None, :]
            kk = np.arange(i0 * KB, (i0 + w) * KB)[:, None]
            if np.array_equal(R.T, qq >= kk):
                key = ('causal', i0 * KB - j * QT, w)
            else:
                Mt = R.T.reshape(w, KB, QT)
                Mt = np.ascontiguousarray(Mt.transpose(1, 0, 2)).reshape(KB, w * QT)
                key = ('data', Mt.tobytes(), w)
            if key not in pat_idx:
                pat_idx[key] = len(pats)
                if key[0] == 'causal':
                    pats.append(('causal', key[1], w))
                else:
                    pats.append(('data', Mt))
            row.append((i0, w, 'mixed', pat_idx[key], y0g, tuple(y0k)))
        plan.append(tuple(row))
    return tuple(plan), pats


def _stack_bias_tiles(pats):
    """Stack 'data' patterns into bias tiles [n, KB, GSIZE*QT] f32."""
    data = [p[1] for p in pats if p[0] == 'data']
    if not data:
        return None
    out = np.full((len(data), KB, GSIZE * QT), B_S, dtype=np.float32)
    for i, m in enumerate(data):
        out[i, :, :m.shape[1]] = np.where(m > 0, B_S, MASKED_B)
    return out


def _build(plan, pats, repeats=1):
    from contextlib import ExitStack

    import concourse.tile as tile
    from concourse import bacc, mybir

    f32 = mybir.dt.float32
    bf16 = mybir.dt.bfloat16
    u16 = mybir.dt.uint16

    nc = bacc.Bacc("TRN2", target_bir_lowering=False, debug=False,
                   num_devices=NCORES)

    qt_d = nc.dram_tensor("qt", [HPC // 2, 128, S], bf16, kind="ExternalInput").ap()
    kt_d = nc.dram_tensor("kt", [HPC // 2, 128, S], bf16, kind="ExternalInput").ap()
    v_d = nc.dram_tensor("v", [HPC, 128, NKB * (D + 1)], bf16,
                         kind="ExternalInput").ap()
    out_d = nc.dram_tensor("out", [HPC, D + 1, S], bf16,
                           kind="ExternalOutput").ap()
    data_idx = {}
    for pi, p in enumerate(pats):
        if p[0] == 'data':
            data_idx[pi] = len(data_idx)
    if data_idx:
        mt_d = nc.dram_tensor("mt", [len(data_idx), KB, GSIZE * QT], f32,
                              kind="ExternalInput").ap()

    # greedy ACT/DVE balance for exp + copy work
    load = {'act': 0.0, 'dve': 0.0}

    def act_exp_cost(cols):
        return (cols + 172) / 1.2 + 57

    def dve_exp_cost(cols):
        return (cols + 120) / 0.96 + 70

    # total qk-group emission sites in one rep (for AV filler spreading)
    sites_per_pair = sum(len(plan[j]) for j in range(NQT))
    total_sites = repeats * (HPC // 2) * sites_per_pair

    with tile.TileContext(nc) as tc, ExitStack() as ctx:
        qk_pool = ctx.enter_context(tc.tile_pool(name="qk", bufs=2))
        v_pool = ctx.enter_context(tc.tile_pool(name="vp", bufs=2))
        # st: ONE tag, 2 allocs/group, bufs=3 -> 3-deep rotation (6 banks)
        # giving ~1.5 groups of slack on the QK^T-vs-exp WAR chain
        st_pool = ctx.enter_context(tc.tile_pool(name="st", bufs=3, space="PSUM"))
        pt_pool = ctx.enter_context(tc.tile_pool(name="pt", bufs=3))
        acc_pool = ctx.enter_context(tc.tile_pool(name="acc", bufs=1, space="PSUM"))
        out_pool = ctx.enter_context(tc.tile_pool(name="ob", bufs=2))
        m_pool = ctx.enter_context(tc.tile_pool(name="mt", bufs=1))

        # bias tiles: B_S where live, MASKED_B where masked
        m_tiles = {}
        for pi, p in enumerate(pats):
            m = m_pool.tile([KB, GSIZE * QT], f32, tag=f"m{pi}", name=f"m{pi}")
            if p[0] == 'causal':
                rel, w = p[1], p[2]
                nc.gpsimd.memset(m[:], float(B_S))
                m3 = m[:].rearrange("p (g y) -> p g y", y=QT)[:, :w, :]
                nc.gpsimd.affine_select(
                    out=m3, in_=m3,
                    compare_op=mybir.AluOpType.is_ge,
                    fill=float(MASKED_B), base=-rel,
                    pattern=[[-KB, w], [1, QT]],
                    channel_multiplier=-1)
            else:
                nc.sync.dma_start(m[:], mt_d[data_idx[pi]])
            m_tiles[pi] = m
        # 0/1 bf16 masks for qtile-0 mixed groups (exact-exp path: those
        # rows would otherwise get 100% Schraudolph weights)
        m01_tiles = {}
        for (i0, w, kind, pat, y0g, y0k) in plan[0]:
            if kind != 'mixed' or pat in m01_tiles or pats[pat][0] != 'causal':
                continue
            rel = pats[pat][1]
            m = m_pool.tile([KB, GSIZE * QT], bf16, tag=f"m01_{pat}",
                            name=f"m01_{pat}")
            nc.gpsimd.memset(m[:], 1.0)
            m3 = m[:].rearrange("p (g y) -> p g y", y=QT)[:, :w, :]
            nc.gpsimd.affine_select(
                out=m3, in_=m3,
                compare_op=mybir.AluOpType.is_ge,
                fill=0.0, base=-rel,
                pattern=[[-KB, w], [1, QT]],
                channel_multiplier=-1)
            m01_tiles[pat] = m

        pending = deque()   # deferred AV/copy/dma closures
        site_counter = [0]  # qk-group sites emitted so far

        def emit_fill():
            # drain pending AV within ~4 qk-group sites so reads stay ahead
            # of the pt/acc buffer rotations (pt bufs=3 covers the slack);
            # only fill on every 2nd site: fewer, larger AV blocks = fewer
            # PE tiling-mode switches (each switch drains the array)
            if site_counter[0] % 2 == 0 and site_counter[0] < total_sites:
                return
            sites_left = total_sites - site_counter[0]
            if sites_left <= 0 or not pending:
                n = len(pending)
            else:
                n = max(6, -(-len(pending) // min(4, sites_left)))
            for _ in range(min(n, len(pending))):
                pending.popleft()()

        for rep in range(repeats):
          for pair in range(HPC // 2):
            kt_c, qt_c = [], []
            v_ts = []
            out_sbs = []
            for c in range(NQT):
                kt1 = qk_pool.tile([128, QT], bf16, tag=f"kt{c}",
                                   name=f"kt{pair}_{c}")
                nc.sync.dma_start(kt1[:], kt_d[pair, :, c * QT:(c + 1) * QT])
                kt_c.append(kt1)
                qt1 = qk_pool.tile([128, QT], bf16, tag=f"qt{c}",
                                   name=f"qt{pair}_{c}")
                nc.sync.dma_start(qt1[:], qt_d[pair, :, c * QT:(c + 1) * QT])
                qt_c.append(qt1)
                if c == 0:
                    for sub in range(2):
                        h = 2 * pair + sub
                        v_t = v_pool.tile([128, NKB * (D + 1)], bf16,
                                          tag=f"v{sub}", name=f"v{h}")
                        nc.sync.dma_start(v_t[:], v_d[h])
                        v_ts.append(v_t)
                        out_sbs.append(
                            out_pool.tile([D + 1, S], bf16, tag=f"o{sub}",
                                          name=f"ob{h}"))

            for j in range(NQT):
                groups = plan[j]
                pt_info = []  # (groups idx -> pt tiles per sub)
                for gi, (i0, w, kind, pat, y0g, y0k) in enumerate(groups):
                    site_counter[0] += 1
                    emit_fill()
                    sts = [st_pool.tile([128, GSIZE * QT], f32, tag="st",
                                        name=f"st{sub}")
                           for sub in range(2)]
                    # paired QK^T: head A on PE rows 0-63, head B on 64-127,
                    # adjacent in the PE queue -> concurrent row tiles
                    for gp in range(w):
                        i = i0 + gp
                        y = y0g  # exp reads [y0g:], so QK^T must cover it
                        for sub in range(2):
                            po = 64 * sub
                            nc.tensor.matmul(
                                sts[sub][:, gp * QT + y:(gp + 1) * QT],
                                lhsT=kt_c[i // 4][po:po + 64,
                                                  (i % 4) * KB:(i % 4 + 1) * KB],
                                rhs=qt_c[j][po:po + 64, y:],
                                start=True, stop=True)
                    pts = []
                    for sub in range(2):
                        st = sts[sub]
                        pt = pt_pool.tile([128, GSIZE * QT], bf16,
                                          tag=f"pt{sub}_{gi}",
                                          name=f"pt{sub}_{gi}")
                        if y0g == 0:
                            st_ap = st[:, :w * QT]
                            pt_ap = pt[:, :w * QT]
                        else:
                            st_ap = st[:].rearrange(
                                "p (g y) -> p g y", y=QT)[:, :w, y0g:]
                            pt_ap = pt[:].rearrange(
                                "p (g y) -> p g y", y=QT)[:, :w, y0g:]
                        cols = st_ap.free_size()
                        if kind == 'mixed' and j == 0 and pat in m01_tiles:
                            # exact path: ACT exp then 0/1 mask mult on DVE
                            m = m01_tiles[pat]
                            if y0g == 0:
                                m_ap = m[:, :w * QT]
                            else:
                                m_ap = m[:].rearrange(
                                    "p (g y) -> p g y", y=QT)[:, :w, y0g:]
                            nc.scalar.activation(
                                pt_ap, st_ap,
                                mybir.ActivationFunctionType.Exp,
                                scale=float(1.0 / np.sqrt(D)))
                            # mask-mult on the otherwise-idle gpsimd engine
                            nc.gpsimd.tensor_mul(pt_ap, pt_ap, m_ap)
                            load['act'] += act_exp_cost(cols)
                        elif kind == 'mixed':
                            m = m_tiles[pat]
                            if y0g == 0:
                                m_ap = m[:, :w * QT]
                            else:
                                m_ap = m[:].rearrange(
                                    "p (g y) -> p g y", y=QT)[:, :w, y0g:]
                            nc.vector.scalar_tensor_tensor(
                                out=pt_ap.bitcast(u16), in0=st_ap,
                                scalar=float(A_S), in1=m_ap,
                                op0=mybir.AluOpType.mult,
                                op1=mybir.AluOpType.add)
                            load['dve'] += dve_exp_cost(cols)
                        elif load['act'] + act_exp_cost(cols) <= \
                                load['dve'] + dve_exp_cost(cols):
                            nc.scalar.activation(
                                pt_ap, st_ap,
                                mybir.ActivationFunctionType.Exp,
                                scale=float(1.0 / np.sqrt(D)))
                            load['act'] += act_exp_cost(cols)
                        else:
                            nc.vector.tensor_scalar(
                                out=pt_ap.bitcast(u16), in0=st_ap,
                                scalar1=float(A_S), scalar2=float(B_S),
                                op0=mybir.AluOpType.mult,
                                op1=mybir.AluOpType.add)
                            load['dve'] += dve_exp_cost(cols)
                        pts.append(pt)
                    pt_info.append(pts)

                # deferred AV for qtile j: full-K=128 matmuls (tile (0,0)),
                # one PSUM bank per head (cross-row-group accumulation into
                # one bank faults on hw, so no k-half row tiling here)
                accs = [acc_pool.tile([D + 1, QT], f32, tag=f"a{sub}",
                                      name=f"a{sub}")
                        for sub in range(2)]
                n_mms = sum(w for (i0, w, *_ ) in groups)

                def mk_mm(sub, gi, gp, first, last, j=j, accs=accs,
                          pt_info=pt_info, groups=groups):
                    i0, w, kind, pat, y0g, y0k = groups[gi]
                    i = i0 + gp
                    y = y0k[gp]
                    pt = pt_info[gi][sub]
                    v_t = v_ts[sub]

                    def mm():
                        nc.tensor.matmul(
                            accs[sub][:, y:],
                            lhsT=v_t[:, i * (D + 1):(i + 1) * (D + 1)],
                            rhs=pt[:, gp * QT + y:(gp + 1) * QT],
                            start=first, stop=last)
                    return mm

                for sub in range(2):
                    mi = 0
                    for gi in range(len(groups)):
                        for gp in range(groups[gi][1]):
                            pending.append(mk_mm(
                                sub, gi, gp,
                                first=(mi == 0), last=(mi == n_mms - 1)))
                            mi += 1

                def mk_tail(sub, j=j, accs=accs, out_sbs=out_sbs, pair=pair):
                    osl = out_sbs[sub][:, j * QT:(j + 1) * QT]

                    def tail():
                        c_act = (512 + 172) / 1.2 + 57
                        c_dve = (512 + 120) / 0.96 + 70
                        if load['act'] + c_act <= load['dve'] + c_dve:
                            nc.scalar.copy(osl, accs[sub][:])
                            load['act'] += c_act
                        else:
                            nc.vector.tensor_copy(osl, accs[sub][:])
                            load['dve'] += c_dve
                        nc.sync.dma_start(
                            out_d[2 * pair + sub, :, j * QT:(j + 1) * QT], osl)
                    return tail

                for sub in range(2):
                    pending.append(mk_tail(sub))

        while pending:
            pending.popleft()()

    nc.compile()
    return nc


def _get_nc(mask):
    key = mask.tobytes()
    if key not in _CACHE:
        plan, pats = _plan_from_mask(mask)
        nc = _build(plan, pats)
        _CACHE[key] = (nc, pats)
    return _CACHE[key]


def _np_bf16():
    from concourse import mybir
    return mybir.dt.np(mybir.dt.bfloat16)


def make_in_map(q, k, v, core):
    """Host-side shard + pack for one core: bf16 qt/kt pairs-on-partitions,
    V augmented with a ones column and swizzled to [KB, NKB*(D+1)]."""
    bf = _np_bf16()
    sl = slice(HPC * core, HPC * (core + 1))
    qc = np.ascontiguousarray(
        q[sl].transpose(0, 2, 1)).reshape(HPC // 2, 128, S).astype(bf)
    kc = np.ascontiguousarray(
        k[sl].transpose(0, 2, 1)).reshape(HPC // 2, 128, S).astype(bf)
    vc = np.concatenate([v[sl], np.ones((HPC, S, 1), dtype=np.float32)], axis=2)
    vc = vc.reshape(HPC, NKB, KB, D + 1).transpose(0, 2, 1, 3)
    vc = np.ascontiguousarray(vc).reshape(HPC, KB, NKB * (D + 1)).astype(bf)
    return {"qt": qc, "kt": kc, "v": vc}


def kernel(q, k, v, mask, _trace=False):
    from concourse.bass_utils import run_bass_kernel_spmd

    mask = np.asarray(mask).astype(bool)
    q = np.asarray(q, dtype=np.float32).reshape(B * H, S, D)
    k = np.asarray(k, dtype=np.float32).reshape(B * H, S, D)
    v = np.asarray(v, dtype=np.float32).reshape(B * H, S, D)

    nc, pats = _get_nc(mask)
    mt = _stack_bias_tiles(pats)

    in_maps = []
    for c in range(NCORES):
        m = make_in_map(q, k, v, c)
        if mt is not None:
            m["mt"] = mt
        in_maps.append(m)

    res = run_bass_kernel_spmd(nc, in_maps, core_ids=list(range(NCORES)),
                               trace=_trace)

    outs = []
    for c in range(NCORES):
        o = np.asarray(res.results[c]["out"], dtype=np.float32)  # [HPC, D+1, S]
        num = o[:, :D, :]
        den = o[:, D:D + 1, :]
        with np.errstate(invalid='ignore', divide='ignore'):
            outs.append((num / den).transpose(0, 2, 1))  # [HPC, S, D]
    full = np.concatenate(outs, axis=0).reshape(B, H, S, D).astype(np.float32)
    if _trace:
        return full, res
    return full


# revision 31
# speedup vs baseline: 1.0990x; 1.0990x over previous
"""Causal attention kernel for Trainium2 (Bass/Tile), 8-core SPMD.

Problem: B=2, H=16, S=2048, D=64 fp32 attention with a causal mask.
Sharding: batch*heads = 32 slices -> 4 heads per core across 8 cores.

Per-core algorithm (heads processed in pairs stacked on SBUF partitions):
  S^T = K @ Q^T blockwise in bf16: [kblock=128, qtile=512] tiles, k on PSUM
  partitions, q on the free dim. The two heads of a pair use PE row tiles
  (0,0)/(64,0) (contraction D=64, auto-derived from base_partition) so
  their QK^T matmuls run CONCURRENTLY in the two halves of the array.
  exp: split across ScalarE (exact ACT exp -> bf16) and VectorE
  (Schraudolph bit-trick: bf16bits = uint16(s*128/(8 ln2) + B)); on the
  DVE path the causal mask is FUSED as an additive per-element bias tile
  (masked -> bits in [2.9k, 5.1k] -> ~1e-30 bf16, avoiding the wrapping
  f32->uint16 convert), one scalar_tensor_tensor op total. qtile-0 groups
  use exact ACT exp + gpsimd mask instead (their rows would otherwise be
  100% Schraudolph-weighted and blow the error budget).
  P^T tiles (bf16) persist in SBUF for a whole qtile; the AV pass
  out^T = V_aug^T @ P^T (V_aug has a ones column -> row 64 = softmax
  denominator) uses full-K=128 matmuls into one PSUM bank per head
  (cross-row-group accumulation into one bank faults on hw). AV matmuls
  are emitted as FILLER between the NEXT qtile's QK^T groups (every 2nd
  group site, to limit PE tiling-mode switches), so PSUM fits exactly:
  st = one tag x bufs=3 rotation (6 banks, ~1.5 groups of WAR slack) +
  2 acc banks. Host divides by the denominator and transposes back.

All matmuls bf16 (1 cycle/row, FWL weight loads); DMA is halved vs fp32.
"""

import sys
from collections import deque

import numpy as np

for _p in ('/opt/trn_rl_repo', '/root/.axon_site/_ro/trn_rl_repo'):
    if _p not in sys.path:
        sys.path.append(_p)

B, H, S, D = 2, 16, 2048, 64
NCORES = 8
HPC = (B * H) // NCORES  # heads per core = 4
QT = 512                 # q tile (PSUM bank free dim)
KB = 128                 # k block (partition dim)
GSIZE = 2                # kblocks per exp group (ACT overhead amortization)
NQT = S // QT            # 4
NKB = S // KB            # 16
YGRAN = 128              # q-trim granularity for matmuls

# Schraudolph exp -> bf16 bits: bits = uint16(s * A_S + B_S), computed on
# raw (unscaled) scores so A_S folds the 1/sqrt(D) softmax scale.
A_S = 128.0 / (float(np.log(2.0)) * 8.0)
C_S = 4.75               # sawtooth centering (covers rint-vs-floor convert)
B_S = 127.0 * 128.0 - C_S
# bias value for masked elements: bits land in [2900, 5100] for any |s|<=47,
# i.e. a ~1e-30 bf16 (effectively 0) without relying on convert saturation
MASKED_B = 4000.0

_CACHE = {}


def _plan_from_mask(mask):
    """Classify each (qtile, kblock-group) region of the mask.

    plan[j]: tuple of groups (i0, w, kind, pat, y0g, y0k):
      kblocks [i0, i0+w), kind in {'full','mixed'}, pat indexes the deduped
      bias patterns, y0g trims the exp AP (256-granular), y0k[gp] trims the
      QK^T / AV matmuls per kblock (YGRAN-granular).
    pats[p]: ('causal', rel, w) generated on-chip, or ('data', arr) DMA'd.
    """
    plan = []
    pats = []
    pat_idx = {}
    for j in range(NQT):
        row = []
        mq = mask[j * QT:(j + 1) * QT]  # [QT, S] (q rows, k cols)
        for i0 in range(0, NKB, GSIZE):
            w = min(GSIZE, NKB - i0)
            blk = [mq[:, i * KB:(i + 1) * KB].any() for i in range(i0, i0 + w)]
            if not any(blk):
                continue
            lead = blk.index(True)
            i0 += lead
            w = len(blk) - lead - blk[::-1].index(True)
            R = mq[:, i0 * KB:(i0 + w) * KB]  # [QT, w*KB]
            y0k = []
            for gp in range(w):
                Rb = R[:, gp * KB:(gp + 1) * KB]
                live = np.nonzero(Rb.any(axis=1))[0]
                y0 = (int(live[0]) // YGRAN) * YGRAN if len(live) else 0
                y0k.append(min(y0, QT - YGRAN))
            if R.all():
                row.append((i0, w, 'full', -1, 0, tuple([0] * w)))
                continue
            y0g = min((y0k[0] // 256) * 256, QT - 256)
            qq = np.arange(j * QT, (j + 1) * QT)[None, :]
            kk = np.arange(i0 * KB, (i0 + w) * KB)[:, None]
            if np.array_equal(R.T, qq >= kk):
                key = ('causal', i0 * KB - j * QT, w)
            else:
                Mt = R.T.reshape(w, KB, QT)
                Mt = np.ascontiguousarray(Mt.transpose(1, 0, 2)).reshape(KB, w * QT)
                key = ('data', Mt.tobytes(), w)
            if key not in pat_idx:
                pat_idx[key] = len(pats)
                if key[0] == 'causal':
                    pats.append(('causal', key[1], w))
                else:
                    pats.append(('data', Mt))
            row.append((i0, w, 'mixed', pat_idx[key], y0g, tuple(y0k)))
        plan.append(tuple(row))
    return tuple(plan), pats


def _stack_bias_tiles(pats):
    """Stack 'data' patterns into bias tiles [n, KB, GSIZE*QT] f32."""
    data = [p[1] for p in pats if p[0] == 'data']
    if not data:
        return None
    out = np.full((len(data), KB, GSIZE * QT), B_S, dtype=np.float32)
    for i, m in enumerate(data):
        out[i, :, :m.shape[1]] = np.where(m > 0, B_S, MASKED_B)
    return out


def _build(plan, pats, repeats=1):
    from contextlib import ExitStack

    import concourse.tile as tile
    from concourse import bacc, mybir

    f32 = mybir.dt.float32
    bf16 = mybir.dt.bfloat16
    u16 = mybir.dt.uint16

    nc = bacc.Bacc("TRN2", target_bir_lowering=False, debug=False,
                   num_devices=NCORES)

    qt_d = nc.dram_tensor("qt", [HPC // 2, 128, S], bf16, kind="ExternalInput").ap()
    kt_d = nc.dram_tensor("kt", [HPC // 2, 128, S], bf16, kind="ExternalInput").ap()
    v_d = nc.dram_tensor("v", [HPC, 128, NKB * (D + 1)], bf16,
                         kind="ExternalInput").ap()
    out_d = nc.dram_tensor("out", [HPC, D + 1, S], bf16,
                           kind="ExternalOutput").ap()
    data_idx = {}
    for pi, p in enumerate(pats):
        if p[0] == 'data':
            data_idx[pi] = len(data_idx)
    if data_idx:
        mt_d = nc.dram_tensor("mt", [len(data_idx), KB, GSIZE * QT], f32,
                              kind="ExternalInput").ap()

    # greedy ACT/DVE balance for exp + copy work
    load = {'act': 0.0, 'dve': 0.0}

    def act_exp_cost(cols):
        return (cols + 172) / 1.2 + 57

    def dve_exp_cost(cols):
        return (cols + 120) / 0.96 + 70

    # total qk-group emission sites in one rep (for AV filler spreading)
    sites_per_pair = sum(len(plan[j]) for j in range(NQT))
    total_sites = repeats * (HPC // 2) * sites_per_pair

    with tile.TileContext(nc) as tc, ExitStack() as ctx:
        qk_pool = ctx.enter_context(tc.tile_pool(name="qk", bufs=2))
        v_pool = ctx.enter_context(tc.tile_pool(name="vp", bufs=2))
        # st: ONE tag, 2 allocs/group, bufs=3 -> 3-deep rotation (6 banks)
        # giving ~1.5 groups of slack on the QK^T-vs-exp WAR chain
        st_pool = ctx.enter_context(tc.tile_pool(name="st", bufs=3, space="PSUM"))
        pt_pool = ctx.enter_context(tc.tile_pool(name="pt", bufs=3))
        acc_pool = ctx.enter_context(tc.tile_pool(name="acc", bufs=1, space="PSUM"))
        out_pool = ctx.enter_context(tc.tile_pool(name="ob", bufs=2))
        m_pool = ctx.enter_context(tc.tile_pool(name="mt", bufs=1))

        # bias tiles: B_S where live, MASKED_B where masked
        m_tiles = {}
        for pi, p in enumerate(pats):
            m = m_pool.tile([KB, GSIZE * QT], f32, tag=f"m{pi}", name=f"m{pi}")
            if p[0] == 'causal':
                rel, w = p[1], p[2]
                nc.gpsimd.memset(m[:], float(B_S))
                m3 = m[:].rearrange("p (g y) -> p g y", y=QT)[:, :w, :]
                nc.gpsimd.affine_select(
                    out=m3, in_=m3,
                    compare_op=mybir.AluOpType.is_ge,
                    fill=float(MASKED_B), base=-rel,
                    pattern=[[-KB, w], [1, QT]],
                    channel_multiplier=-1)
            else:
                nc.sync.dma_start(m[:], mt_d[data_idx[pi]])
            m_tiles[pi] = m
        # 0/1 bf16 masks for qtile-0 mixed groups (exact-exp path: those
        # rows would otherwise get 100% Schraudolph weights)
        m01_tiles = {}
        for (i0, w, kind, pat, y0g, y0k) in plan[0]:
            if kind != 'mixed' or pat in m01_tiles or pats[pat][0] != 'causal':
                continue
            rel = pats[pat][1]
            m = m_pool.tile([KB, GSIZE * QT], bf16, tag=f"m01_{pat}",
                            name=f"m01_{pat}")
            nc.gpsimd.memset(m[:], 1.0)
            m3 = m[:].rearrange("p (g y) -> p g y", y=QT)[:, :w, :]
            nc.gpsimd.affine_select(
                out=m3, in_=m3,
                compare_op=mybir.AluOpType.is_ge,
                fill=0.0, base=-rel,
                pattern=[[-KB, w], [1, QT]],
                channel_multiplier=-1)
            m01_tiles[pat] = m

        pending = deque()   # deferred AV/copy/dma closures
        site_counter = [0]  # qk-group sites emitted so far

        def emit_fill():
            # drain pending AV within ~4 qk-group sites so reads stay ahead
            # of the pt/acc buffer rotations (pt bufs=3 covers the slack);
            # only fill on every 2nd site: fewer, larger AV blocks = fewer
            # PE tiling-mode switches (each switch drains the array)
            if site_counter[0] % 2 == 0 and site_counter[0] < total_sites:
                return
            sites_left = total_sites - site_counter[0]
            if sites_left <= 0 or not pending:
                n = len(pending)
            else:
                n = max(6, -(-len(pending) // min(4, sites_left)))
            for _ in range(min(n, len(pending))):
                pending.popleft()()

        for rep in range(repeats):
          for pair in range(HPC // 2):
            kt_c, qt_c = [], []
            v_ts = []
            out_sbs = []
            for c in range(NQT):
                kt1 = qk_pool.tile([128, QT], bf16, tag=f"kt{c}",
                                   name=f"kt{pair}_{c}")
                nc.sync.dma_start(kt1[:], kt_d[pair, :, c * QT:(c + 1) * QT])
                kt_c.append(kt1)
                qt1 = qk_pool.tile([128, QT], bf16, tag=f"qt{c}",
                                   name=f"qt{pair}_{c}")
                nc.sync.dma_start(qt1[:], qt_d[pair, :, c * QT:(c + 1) * QT])
                qt_c.append(qt1)
                if c == 0:
                    for sub in range(2):
                        h = 2 * pair + sub
                        v_t = v_pool.tile([128, NKB * (D + 1)], bf16,
                                          tag=f"v{sub}", name=f"v{h}")
                        nc.sync.dma_start(v_t[:], v_d[h])
                        v_ts.append(v_t)
                        out_sbs.append(
                            out_pool.tile([D + 1, S], bf16, tag=f"o{sub}",
                                          name=f"ob{h}"))

            for j in range(NQT):
                groups = plan[j]
                pt_info = []  # (groups idx -> pt tiles per sub)
                for gi, (i0, w, kind, pat, y0g, y0k) in enumerate(groups):
                    site_counter[0] += 1
                    emit_fill()
                    sts = [st_pool.tile([128, GSIZE * QT], f32, tag="st",
                                        name=f"st{sub}")
                           for sub in range(2)]
                    # paired QK^T: head A on PE rows 0-63, head B on 64-127,
                    # adjacent in the PE queue -> concurrent row tiles
                    for gp in range(w):
                        i = i0 + gp
                        y = y0g  # exp reads [y0g:], so QK^T must cover it
                        for sub in range(2):
                            po = 64 * sub
                            nc.tensor.matmul(
                                sts[sub][:, gp * QT + y:(gp + 1) * QT],
                                lhsT=kt_c[i // 4][po:po + 64,
                                                  (i % 4) * KB:(i % 4 + 1) * KB],
                                rhs=qt_c[j][po:po + 64, y:],
                                start=True, stop=True)
                    pts = []
                    for sub in range(2):
                        st = sts[sub]
                        pt = pt_pool.tile([128, GSIZE * QT], bf16,
                                          tag=f"pt{sub}_{gi}",
                                          name=f"pt{sub}_{gi}")
                        if y0g == 0:
                            st_ap = st[:, :w * QT]
                            pt_ap = pt[:, :w * QT]
                        else:
                            st_ap = st[:].rearrange(
                                "p (g y) -> p g y", y=QT)[:, :w, y0g:]
                            pt_ap = pt[:].rearrange(
                                "p (g y) -> p g y", y=QT)[:, :w, y0g:]
                        cols = st_ap.free_size()
                        if kind == 'mixed' and j == 0 and pat in m01_tiles:
                            # exact path: ACT exp then 0/1 mask mult on DVE
                            m = m01_tiles[pat]
                            if y0g == 0:
                                m_ap = m[:, :w * QT]
                            else:
                                m_ap = m[:].rearrange(
                                    "p (g y) -> p g y", y=QT)[:, :w, y0g:]
                            nc.scalar.activation(
                                pt_ap, st_ap,
                                mybir.ActivationFunctionType.Exp,
                                scale=float(1.0 / np.sqrt(D)))
                            # mask-mult on the otherwise-idle gpsimd engine
                            nc.gpsimd.tensor_mul(pt_ap, pt_ap, m_ap)
                            load['act'] += act_exp_cost(cols)
                        elif kind == 'mixed':
                            m = m_tiles[pat]
                            if y0g == 0:
                                m_ap = m[:, :w * QT]
                            else:
                                m_ap = m[:].rearrange(
                                    "p (g y) -> p g y", y=QT)[:, :w, y0g:]
                            nc.vector.scalar_tensor_tensor(
                                out=pt_ap.bitcast(u16), in0=st_ap,
                                scalar=float(A_S), in1=m_ap,
                                op0=mybir.AluOpType.mult,
                                op1=mybir.AluOpType.add)
                            load['dve'] += dve_exp_cost(cols)
                        elif load['act'] + act_exp_cost(cols) <= \
                                load['dve'] + dve_exp_cost(cols):
                            nc.scalar.activation(
                                pt_ap, st_ap,
                                mybir.ActivationFunctionType.Exp,
                                scale=float(1.0 / np.sqrt(D)))
                            load['act'] += act_exp_cost(cols)
                        else:
                            nc.vector.tensor_scalar(
                                out=pt_ap.bitcast(u16), in0=st_ap,
                                scalar1=float(A_S), scalar2=float(B_S),
                                op0=mybir.AluOpType.mult,
                                op1=mybir.AluOpType.add)
                            load['dve'] += dve_exp_cost(cols)
                        pts.append(pt)
                    pt_info.append(pts)

                # deferred AV for qtile j: full-K=128 matmuls (tile (0,0)),
                # one PSUM bank per head (cross-row-group accumulation into
                # one bank faults on hw, so no k-half row tiling here)
                accs = [acc_pool.tile([D + 1, QT], f32, tag=f"a{sub}",
                                      name=f"a{sub}")
                        for sub in range(2)]
                n_mms = sum(w for (i0, w, *_ ) in groups)

                def mk_mm(sub, gi, gp, first, last, j=j, accs=accs,
                          pt_info=pt_info, groups=groups):
                    i0, w, kind, pat, y0g, y0k = groups[gi]
                    i = i0 + gp
                    y = y0k[gp]
                    pt = pt_info[gi][sub]
                    v_t = v_ts[sub]

                    def mm():
                        nc.tensor.matmul(
                            accs[sub][:, y:],
                            lhsT=v_t[:, i * (D + 1):(i + 1) * (D + 1)],
                            rhs=pt[:, gp * QT + y:(gp + 1) * QT],
                            start=first, stop=last)
                    return mm

                for sub in range(2):
                    mi = 0
                    for gi in range(len(groups)):
                        for gp in range(groups[gi][1]):
                            pending.append(mk_mm(
                                sub, gi, gp,
                                first=(mi == 0), last=(mi == n_mms - 1)))
                            mi += 1

                def mk_tail(sub, j=j, accs=accs, out_sbs=out_sbs, pair=pair):
                    osl = out_sbs[sub][:, j * QT:(j + 1) * QT]

                    def tail():
                        c_act = (512 + 172) / 1.2 + 57
                        c_dve = (512 + 120) / 0.96 + 70
                        if load['act'] + c_act <= load['dve'] + c_dve:
                            nc.scalar.copy(osl, accs[sub][:])
                            load['act'] += c_act
                        else:
                            nc.vector.tensor_copy(osl, accs[sub][:])
                            load['dve'] += c_dve
                        nc.sync.dma_start(
                            out_d[2 * pair + sub, :, j * QT:(j + 1) * QT], osl)
                    return tail

                for sub in range(2):
                    pending.append(mk_tail(sub))

        while pending:
            pending.popleft()()

    nc.compile()
    return nc


def _get_nc(mask):
    key = mask.tobytes()
    if key not in _CACHE:
        plan, pats = _plan_from_mask(mask)
        nc = _build(plan, pats)
        _CACHE[key] = (nc, pats)
    return _CACHE[key]


def _np_bf16():
    from concourse import mybir
    return mybir.dt.np(mybir.dt.bfloat16)


def make_in_map(q, k, v, core):
    """Host-side shard + pack for one core: bf16 qt/kt pairs-on-partitions,
    V augmented with a ones column and swizzled to [KB, NKB*(D+1)]."""
    bf = _np_bf16()
    sl = slice(HPC * core, HPC * (core + 1))
    qc = np.ascontiguousarray(
        q[sl].transpose(0, 2, 1)).reshape(HPC // 2, 128, S).astype(bf)
    kc = np.ascontiguousarray(
        k[sl].transpose(0, 2, 1)).reshape(HPC // 2, 128, S).astype(bf)
    vc = np.concatenate([v[sl], np.ones((HPC, S, 1), dtype=np.float32)], axis=2)
    vc = vc.reshape(HPC, NKB, KB, D + 1).transpose(0, 2, 1, 3)
    vc = np.ascontiguousarray(vc).reshape(HPC, KB, NKB * (D + 1)).astype(bf)
    return {"qt": qc, "kt": kc, "v": vc}


def kernel(q, k, v, mask, _trace=False):
    from concourse.bass_utils import run_bass_kernel_spmd

    mask = np.asarray(mask).astype(bool)
    q = np.asarray(q, dtype=np.float32).reshape(B * H, S, D)
    k = np.asarray(k, dtype=np.float32).reshape(B * H, S, D)
    v = np.asarray(v, dtype=np.float32).reshape(B * H, S, D)

    nc, pats = _get_nc(mask)
    mt = _stack_bias_tiles(pats)

    in_maps = []
    for c in range(NCORES):
        m = make_in_map(q, k, v, c)
        if mt is not None:
            m["mt"] = mt
        in_maps.append(m)

    res = run_bass_kernel_spmd(nc, in_maps, core_ids=list(range(NCORES)),
                               trace=_trace)

    outs = []
    for c in range(NCORES):
        o = np.asarray(res.results[c]["out"], dtype=np.float32)  # [HPC, D+1, S]
        num = o[:, :D, :]
        den = o[:, D:D + 1, :]
        with np.errstate(invalid='ignore', divide='ignore'):
            outs.append((num / den).transpose(0, 2, 1))  # [HPC, S, D]
    full = np.concatenate(outs, axis=0).reshape(B, H, S, D).astype(np.float32)
    if _trace:
        return full, res
    return full


# revision 35
# speedup vs baseline: 1.2261x; 1.1156x over previous
"""Causal attention kernel for Trainium2 (Bass/Tile), 8-core SPMD.

Problem: B=2, H=16, S=2048, D=64 fp32 attention with a causal mask.
Sharding: batch*heads = 32 slices -> 4 heads per core across 8 cores.

Per-core algorithm (heads processed in pairs stacked on SBUF partitions):
  S^T = K @ Q^T blockwise in bf16: [kblock=128, qtile=512] tiles, k on PSUM
  partitions, q on the free dim. The two heads of a pair use PE row tiles
  (0,0)/(64,0) (contraction D=64, auto-derived from base_partition) so
  their QK^T matmuls run CONCURRENTLY in the two halves of the array.
  exp: split across ScalarE (exact ACT exp -> bf16) and VectorE
  (Schraudolph bit-trick: bf16bits = uint16(s*128/(8 ln2) + B)); on the
  DVE path the causal mask is FUSED as an additive per-element bias tile
  (masked -> bits in [2.9k, 5.1k] -> ~1e-30 bf16, avoiding the wrapping
  f32->uint16 convert), one scalar_tensor_tensor op total. qtile-0 groups
  use exact ACT exp + gpsimd mask instead (their rows would otherwise be
  100% Schraudolph-weighted and blow the error budget).
  P^T tiles (bf16) persist in SBUF for a whole qtile; the AV pass
  out^T = V_aug^T @ P^T (V_aug has a ones column -> row 64 = softmax
  denominator) uses full-K=128 matmuls into one PSUM bank per head
  (cross-row-group accumulation into one bank faults on hw). AV matmuls
  are emitted as FILLER between the NEXT qtile's QK^T groups, so PSUM
  fits exactly:
  st = one tag x bufs=3 rotation (6 banks, ~1.5 groups of WAR slack) +
  2 acc banks. Host divides by the denominator and transposes back.

All matmuls bf16 (1 cycle/row, FWL weight loads); DMA is halved vs fp32.
"""

import sys
from collections import deque

import numpy as np

for _p in ('/opt/trn_rl_repo', '/root/.axon_site/_ro/trn_rl_repo'):
    if _p not in sys.path:
        sys.path.append(_p)

B, H, S, D = 2, 16, 2048, 64
NCORES = 8
HPC = (B * H) // NCORES  # heads per core = 4
QT = 512                 # q tile (PSUM bank free dim)
KB = 128                 # k block (partition dim)
GSIZE = 2                # kblocks per exp group (ACT overhead amortization)
NQT = S // QT            # 4
NKB = S // KB            # 16
YGRAN = 128              # q-trim granularity for matmuls

# Schraudolph exp -> bf16 bits: bits = uint16(s * A_S + B_S), computed on
# raw (unscaled) scores so A_S folds the 1/sqrt(D) softmax scale.
A_S = 128.0 / (float(np.log(2.0)) * 8.0)
C_S = 4.75               # sawtooth centering (covers rint-vs-floor convert)
B_S = 127.0 * 128.0 - C_S
# bias value for masked elements: bits land in [2900, 5100] for any |s|<=47,
# i.e. a ~1e-30 bf16 (effectively 0) without relying on convert saturation
MASKED_B = 4000.0
FILL_EVERY = 1           # AV-filler batching: every Nth qk-group site
                         # (r128 slope: every-1 63.5us vs every-2 69.2us —
                         # st-WAR stalls outweigh PE mode-switch drains)

_CACHE = {}


def _plan_from_mask(mask):
    """Classify each (qtile, kblock-group) region of the mask.

    plan[j]: tuple of groups (i0, w, kind, pat, y0g, y0k):
      kblocks [i0, i0+w), kind in {'full','mixed'}, pat indexes the deduped
      bias patterns, y0g trims the exp AP (256-granular), y0k[gp] trims the
      QK^T / AV matmuls per kblock (YGRAN-granular).
    pats[p]: ('causal', rel, w) generated on-chip, or ('data', arr) DMA'd.
    """
    plan = []
    pats = []
    pat_idx = {}
    for j in range(NQT):
        row = []
        mq = mask[j * QT:(j + 1) * QT]  # [QT, S] (q rows, k cols)
        for i0 in range(0, NKB, GSIZE):
            w = min(GSIZE, NKB - i0)
            blk = [mq[:, i * KB:(i + 1) * KB].any() for i in range(i0, i0 + w)]
            if not any(blk):
                continue
            lead = blk.index(True)
            i0 += lead
            w = len(blk) - lead - blk[::-1].index(True)
            R = mq[:, i0 * KB:(i0 + w) * KB]  # [QT, w*KB]
            y0k = []
            for gp in range(w):
                Rb = R[:, gp * KB:(gp + 1) * KB]
                live = np.nonzero(Rb.any(axis=1))[0]
                y0 = (int(live[0]) // YGRAN) * YGRAN if len(live) else 0
                y0k.append(min(y0, QT - YGRAN))
            if R.all():
                row.append((i0, w, 'full', -1, 0, tuple([0] * w)))
                continue
            y0g = min((y0k[0] // 256) * 256, QT - 256)
            qq = np.arange(j * QT, (j + 1) * QT)[None, :]
            kk = np.arange(i0 * KB, (i0 + w) * KB)[:, None]
            if np.array_equal(R.T, qq >= kk):
                key = ('causal', i0 * KB - j * QT, w)
            else:
                Mt = R.T.reshape(w, KB, QT)
                Mt = np.ascontiguousarray(Mt.transpose(1, 0, 2)).reshape(KB, w * QT)
                key = ('data', Mt.tobytes(), w)
            if key not in pat_idx:
                pat_idx[key] = len(pats)
                if key[0] == 'causal':
                    pats.append(('causal', key[1], w))
                else:
                    pats.append(('data', Mt))
            row.append((i0, w, 'mixed', pat_idx[key], y0g, tuple(y0k)))
        plan.append(tuple(row))
    return tuple(plan), pats


def _stack_bias_tiles(pats):
    """Stack 'data' patterns into bias tiles [n, KB, GSIZE*QT] f32."""
    data = [p[1] for p in pats if p[0] == 'data']
    if not data:
        return None
    out = np.full((len(data), KB, GSIZE * QT), B_S, dtype=np.float32)
    for i, m in enumerate(data):
        out[i, :, :m.shape[1]] = np.where(m > 0, B_S, MASKED_B)
    return out


def _build(plan, pats, repeats=1):
    from contextlib import ExitStack

    import concourse.tile as tile
    from concourse import bacc, mybir

    f32 = mybir.dt.float32
    bf16 = mybir.dt.bfloat16
    u16 = mybir.dt.uint16

    nc = bacc.Bacc("TRN2", target_bir_lowering=False, debug=False,
                   num_devices=NCORES)

    qt_d = nc.dram_tensor("qt", [HPC // 2, 128, S], bf16, kind="ExternalInput").ap()
    kt_d = nc.dram_tensor("kt", [HPC // 2, 128, S], bf16, kind="ExternalInput").ap()
    v_d = nc.dram_tensor("v", [HPC, 128, NKB * (D + 1)], bf16,
                         kind="ExternalInput").ap()
    out_d = nc.dram_tensor("out", [HPC, D + 1, S], bf16,
                           kind="ExternalOutput").ap()
    data_idx = {}
    for pi, p in enumerate(pats):
        if p[0] == 'data':
            data_idx[pi] = len(data_idx)
    if data_idx:
        mt_d = nc.dram_tensor("mt", [len(data_idx), KB, GSIZE * QT], f32,
                              kind="ExternalInput").ap()

    # greedy ACT/DVE balance for exp + copy work
    load = {'act': 0.0, 'dve': 0.0}

    def act_exp_cost(cols):
        return (cols + 172) / 1.2 + 57

    def dve_exp_cost(cols):
        return (cols + 120) / 0.96 + 70

    # total qk-group emission sites in one rep (for AV filler spreading)
    sites_per_pair = sum(len(plan[j]) for j in range(NQT))
    total_sites = repeats * (HPC // 2) * sites_per_pair

    with tile.TileContext(nc) as tc, ExitStack() as ctx:
        qk_pool = ctx.enter_context(tc.tile_pool(name="qk", bufs=2))
        v_pool = ctx.enter_context(tc.tile_pool(name="vp", bufs=2))
        # st: ONE tag, 2 allocs/group, bufs=3 -> 3-deep rotation (6 banks)
        # giving ~1.5 groups of slack on the QK^T-vs-exp WAR chain
        st_pool = ctx.enter_context(tc.tile_pool(name="st", bufs=3, space="PSUM"))
        pt_pool = ctx.enter_context(tc.tile_pool(name="pt", bufs=3))
        acc_pool = ctx.enter_context(tc.tile_pool(name="acc", bufs=1, space="PSUM"))
        out_pool = ctx.enter_context(tc.tile_pool(name="ob", bufs=2))
        m_pool = ctx.enter_context(tc.tile_pool(name="mt", bufs=1))

        # bias tiles: B_S where live, MASKED_B where masked
        m_tiles = {}
        for pi, p in enumerate(pats):
            m = m_pool.tile([KB, GSIZE * QT], f32, tag=f"m{pi}", name=f"m{pi}")
            if p[0] == 'causal':
                rel, w = p[1], p[2]
                nc.gpsimd.memset(m[:], float(B_S))
                m3 = m[:].rearrange("p (g y) -> p g y", y=QT)[:, :w, :]
                nc.gpsimd.affine_select(
                    out=m3, in_=m3,
                    compare_op=mybir.AluOpType.is_ge,
                    fill=float(MASKED_B), base=-rel,
                    pattern=[[-KB, w], [1, QT]],
                    channel_multiplier=-1)
            else:
                nc.sync.dma_start(m[:], mt_d[data_idx[pi]])
            m_tiles[pi] = m
        # 0/1 bf16 masks for qtile-0 mixed groups (exact-exp path: those
        # rows would otherwise get 100% Schraudolph weights)
        m01_tiles = {}
        for (i0, w, kind, pat, y0g, y0k) in plan[0]:
            if kind != 'mixed' or pat in m01_tiles or pats[pat][0] != 'causal':
                continue
            rel = pats[pat][1]
            m = m_pool.tile([KB, GSIZE * QT], bf16, tag=f"m01_{pat}",
                            name=f"m01_{pat}")
            nc.gpsimd.memset(m[:], 1.0)
            m3 = m[:].rearrange("p (g y) -> p g y", y=QT)[:, :w, :]
            nc.gpsimd.affine_select(
                out=m3, in_=m3,
                compare_op=mybir.AluOpType.is_ge,
                fill=0.0, base=-rel,
                pattern=[[-KB, w], [1, QT]],
                channel_multiplier=-1)
            m01_tiles[pat] = m

        pending = deque()   # deferred AV/copy/dma closures
        site_counter = [0]  # qk-group sites emitted so far

        def emit_fill():
            # drain pending AV within ~4 qk-group sites so reads stay ahead
            # of the pt/acc buffer rotations (pt bufs=3 covers the slack);
            # only fill on every 2nd site: fewer, larger AV blocks = fewer
            # PE tiling-mode switches (each switch drains the array)
            if FILL_EVERY > 1 and site_counter[0] % FILL_EVERY != 1 \
                    and site_counter[0] < total_sites:
                return
            sites_left = total_sites - site_counter[0]
            if sites_left <= 0 or not pending:
                n = len(pending)
            else:
                n = max(3 * FILL_EVERY,
                        -(-len(pending) // min(4, sites_left)))
            for _ in range(min(n, len(pending))):
                pending.popleft()()

        for rep in range(repeats):
          for pair in range(HPC // 2):
            kt_c, qt_c = [], []
            v_ts = []
            out_sbs = []
            for c in range(NQT):
                kt1 = qk_pool.tile([128, QT], bf16, tag=f"kt{c}",
                                   name=f"kt{pair}_{c}")
                nc.sync.dma_start(kt1[:], kt_d[pair, :, c * QT:(c + 1) * QT])
                kt_c.append(kt1)
                qt1 = qk_pool.tile([128, QT], bf16, tag=f"qt{c}",
                                   name=f"qt{pair}_{c}")
                nc.sync.dma_start(qt1[:], qt_d[pair, :, c * QT:(c + 1) * QT])
                qt_c.append(qt1)
                if c == 0:
                    for sub in range(2):
                        h = 2 * pair + sub
                        v_t = v_pool.tile([128, NKB * (D + 1)], bf16,
                                          tag=f"v{sub}", name=f"v{h}")
                        nc.sync.dma_start(v_t[:], v_d[h])
                        v_ts.append(v_t)
                        out_sbs.append(
                            out_pool.tile([D + 1, S], bf16, tag=f"o{sub}",
                                          name=f"ob{h}"))

            for j in range(NQT):
                groups = plan[j]
                pt_info = []  # (groups idx -> pt tiles per sub)
                for gi, (i0, w, kind, pat, y0g, y0k) in enumerate(groups):
                    site_counter[0] += 1
                    emit_fill()
                    sts = [st_pool.tile([128, GSIZE * QT], f32, tag="st",
                                        name=f"st{sub}")
                           for sub in range(2)]
                    # paired QK^T: head A on PE rows 0-63, head B on 64-127,
                    # adjacent in the PE queue -> concurrent row tiles
                    for gp in range(w):
                        i = i0 + gp
                        y = y0g  # exp reads [y0g:], so QK^T must cover it
                        for sub in range(2):
                            po = 64 * sub
                            nc.tensor.matmul(
                                sts[sub][:, gp * QT + y:(gp + 1) * QT],
                                lhsT=kt_c[i // 4][po:po + 64,
                                                  (i % 4) * KB:(i % 4 + 1) * KB],
                                rhs=qt_c[j][po:po + 64, y:],
                                start=True, stop=True)
                    pts = []
                    for sub in range(2):
                        st = sts[sub]
                        pt = pt_pool.tile([128, GSIZE * QT], bf16,
                                          tag=f"pt{sub}_{gi}",
                                          name=f"pt{sub}_{gi}")
                        if y0g == 0:
                            st_ap = st[:, :w * QT]
                            pt_ap = pt[:, :w * QT]
                        else:
                            st_ap = st[:].rearrange(
                                "p (g y) -> p g y", y=QT)[:, :w, y0g:]
                            pt_ap = pt[:].rearrange(
                                "p (g y) -> p g y", y=QT)[:, :w, y0g:]
                        cols = st_ap.free_size()
                        if kind == 'mixed' and j == 0 and pat in m01_tiles:
                            # exact path: ACT exp then 0/1 mask mult on DVE
                            m = m01_tiles[pat]
                            if y0g == 0:
                                m_ap = m[:, :w * QT]
                            else:
                                m_ap = m[:].rearrange(
                                    "p (g y) -> p g y", y=QT)[:, :w, y0g:]
                            nc.scalar.activation(
                                pt_ap, st_ap,
                                mybir.ActivationFunctionType.Exp,
                                scale=float(1.0 / np.sqrt(D)))
                            # mask-mult on the otherwise-idle gpsimd engine
                            nc.gpsimd.tensor_mul(pt_ap, pt_ap, m_ap)
                            load['act'] += act_exp_cost(cols)
                        elif kind == 'mixed':
                            m = m_tiles[pat]
                            if y0g == 0:
                                m_ap = m[:, :w * QT]
                            else:
                                m_ap = m[:].rearrange(
                                    "p (g y) -> p g y", y=QT)[:, :w, y0g:]
                            nc.vector.scalar_tensor_tensor(
                                out=pt_ap.bitcast(u16), in0=st_ap,
                                scalar=float(A_S), in1=m_ap,
                                op0=mybir.AluOpType.mult,
                                op1=mybir.AluOpType.add)
                            load['dve'] += dve_exp_cost(cols)
                        elif load['act'] + act_exp_cost(cols) <= \
                                load['dve'] + dve_exp_cost(cols):
                            nc.scalar.activation(
                                pt_ap, st_ap,
                                mybir.ActivationFunctionType.Exp,
                                scale=float(1.0 / np.sqrt(D)))
                            load['act'] += act_exp_cost(cols)
                        else:
                            nc.vector.tensor_scalar(
                                out=pt_ap.bitcast(u16), in0=st_ap,
                                scalar1=float(A_S), scalar2=float(B_S),
                                op0=mybir.AluOpType.mult,
                                op1=mybir.AluOpType.add)
                            load['dve'] += dve_exp_cost(cols)
                        pts.append(pt)
                    pt_info.append(pts)

                # deferred AV for qtile j: full-K=128 matmuls (tile (0,0)),
                # one PSUM bank per head (cross-row-group accumulation into
                # one bank faults on hw, so no k-half row tiling here)
                accs = [acc_pool.tile([D + 1, QT], f32, tag=f"a{sub}",
                                      name=f"a{sub}")
                        for sub in range(2)]
                n_mms = sum(w for (i0, w, *_ ) in groups)

                def mk_mm(sub, gi, gp, first, last, j=j, accs=accs,
                          pt_info=pt_info, groups=groups):
                    i0, w, kind, pat, y0g, y0k = groups[gi]
                    i = i0 + gp
                    y = y0k[gp]
                    pt = pt_info[gi][sub]
                    v_t = v_ts[sub]

                    def mm():
                        nc.tensor.matmul(
                            accs[sub][:, y:],
                            lhsT=v_t[:, i * (D + 1):(i + 1) * (D + 1)],
                            rhs=pt[:, gp * QT + y:(gp + 1) * QT],
                            start=first, stop=last)
                    return mm

                for sub in range(2):
                    mi = 0
                    for gi in range(len(groups)):
                        for gp in range(groups[gi][1]):
                            pending.append(mk_mm(
                                sub, gi, gp,
                                first=(mi == 0), last=(mi == n_mms - 1)))
                            mi += 1

                def mk_tail(sub, j=j, accs=accs, out_sbs=out_sbs, pair=pair):
                    osl = out_sbs[sub][:, j * QT:(j + 1) * QT]

                    def tail():
                        c_act = (512 + 172) / 1.2 + 57
                        c_dve = (512 + 120) / 0.96 + 70
                        if load['act'] + c_act <= load['dve'] + c_dve:
                            nc.scalar.copy(osl, accs[sub][:])
                            load['act'] += c_act
                        else:
                            nc.vector.tensor_copy(osl, accs[sub][:])
                            load['dve'] += c_dve
                        nc.sync.dma_start(
                            out_d[2 * pair + sub, :, j * QT:(j + 1) * QT], osl)
                    return tail

                for sub in range(2):
                    pending.append(mk_tail(sub))

        while pending:
            pending.popleft()()

    nc.compile()
    return nc


def _get_nc(mask):
    key = mask.tobytes()
    if key not in _CACHE:
        plan, pats = _plan_from_mask(mask)
        nc = _build(plan, pats)
        _CACHE[key] = (nc, pats)
    return _CACHE[key]


def _np_bf16():
    from concourse import mybir
    return mybir.dt.np(mybir.dt.bfloat16)


def make_in_map(q, k, v, core):
    """Host-side shard + pack for one core: bf16 qt/kt pairs-on-partitions,
    V augmented with a ones column and swizzled to [KB, NKB*(D+1)]."""
    bf = _np_bf16()
    sl = slice(HPC * core, HPC * (core + 1))
    qc = np.ascontiguousarray(
        q[sl].transpose(0, 2, 1)).reshape(HPC // 2, 128, S).astype(bf)
    kc = np.ascontiguousarray(
        k[sl].transpose(0, 2, 1)).reshape(HPC // 2, 128, S).astype(bf)
    vc = np.concatenate([v[sl], np.ones((HPC, S, 1), dtype=np.float32)], axis=2)
    vc = vc.reshape(HPC, NKB, KB, D + 1).transpose(0, 2, 1, 3)
    vc = np.ascontiguousarray(vc).reshape(HPC, KB, NKB * (D + 1)).astype(bf)
    return {"qt": qc, "kt": kc, "v": vc}


def kernel(q, k, v, mask, _trace=False):
    from concourse.bass_utils import run_bass_kernel_spmd

    mask = np.asarray(mask).astype(bool)
    q = np.asarray(q, dtype=np.float32).reshape(B * H, S, D)
    k = np.asarray(k, dtype=np.float32).reshape(B * H, S, D)
    v = np.asarray(v, dtype=np.float32).reshape(B * H, S, D)

    nc, pats = _get_nc(mask)
    mt = _stack_bias_tiles(pats)

    in_maps = []
    for c in range(NCORES):
        m = make_in_map(q, k, v, c)
        if mt is not None:
            m["mt"] = mt
        in_maps.append(m)

    res = run_bass_kernel_spmd(nc, in_maps, core_ids=list(range(NCORES)),
                               trace=_trace)

    outs = []
    for c in range(NCORES):
        o = np.asarray(res.results[c]["out"], dtype=np.float32)  # [HPC, D+1, S]
        num = o[:, :D, :]
        den = o[:, D:D + 1, :]
        with np.errstate(invalid='ignore', divide='ignore'):
            outs.append((num / den).transpose(0, 2, 1))  # [HPC, S, D]
    full = np.concatenate(outs, axis=0).reshape(B, H, S, D).astype(np.float32)
    if _trace:
        return full, res
    return full
